# revision 39
# baseline (speedup 1.0000x reference)
"""Causal self-attention (GQA + RoPE) Trainium2 kernel.

Full-input contract: kernel(**inputs) takes the unsharded tensors and returns
the full [B, T, C] output. Internally shards over 8 NeuronCores as
(batch b in {0,1}) x (kv-head group g in {0..3}); each core computes the
attention output of its 4 query heads (one kv head) for its batch and the
partial out-projection against its 512 rows of Wo. The host sums the 4 group
partials per batch.

Per-core dataflow: projection (tci) and attention (qb=tci) segments are
interleaved; causality guarantees attention block qb only needs K/V/Q through
t-chunk qb. Projection runs one output at a time (Q0, K, V, Q1..Q3) so the
rope chain for the first attention head starts 1/6 into the segment instead
of at its end. Q/K stay f32r (fast DVE writes, f32r x f32r S-matmul);
x/weights/V/P/out-proj are bf16. Host pre-shuffles all inputs into SBUF
partition layout so every DMA is a contiguous 2D pattern, spread over the
sync / scalar / gpsimd DMA rings to match when each tensor is needed.
Softmax denominators are running sums on DVE/GpSimd (pacc) + one
ones-matmul per (head, q-block). S-matmuls are software-pipelined one k-tile
ahead of AV per head so exp latency stays off the PE critical path. The
rotate-half is fused into the rope sin-muls via partition-offset reads of a
host-rolled sin table.
"""

import sys

for _p in ("/opt/trn_rl_repo", "/root/.axon_site/_ro/trn_rl_repo"):
    if _p not in sys.path:
        sys.path.append(_p)

import numpy as np
import ml_dtypes
from contextlib import ExitStack

import concourse.bass as bass
import concourse.bacc as bacc
import concourse.tile as tile
import concourse.mybir as mybir
from concourse.bass_utils import run_bass_kernel_spmd

F32 = mybir.dt.float32
F32R = mybir.dt.float32r
BF16 = mybir.dt.bfloat16

B, T, C = 2, 2048, 2048
N_HEADS, N_KV_HEADS, HD = 16, 4, 128
G = N_HEADS // N_KV_HEADS  # heads per group = 4
GW = G * HD  # 512, per-group Q width / Wo row count
N_CORES = 8
TC = 512  # q-block width
NTC = T // TC  # 4
NKT = T // HD  # 16 k-tiles of 128
NCC = C // 128  # 16 contraction chunks
MASK_NEG = -1.0e30

_prog_cache = {}


def _build_program():
    nc = bacc.Bacc(
        "TRN2",
        target_bir_lowering=False,
        debug=False,
        enable_asserts=False,
        num_devices=N_CORES,
    )

    # all host tensors pre-shuffled to SBUF layout (partition dim first);
    # wq additionally grouped by head: [128, (j, ci, d)]
    xh = nc.dram_tensor("xh", [128, NTC * NCC * TC], BF16, kind="ExternalInput").ap()
    wq = nc.dram_tensor("wq", [128, G * NCC * HD], BF16, kind="ExternalInput").ap()
    wk = nc.dram_tensor("wk", [128, NCC * HD], BF16, kind="ExternalInput").ap()
    wv = nc.dram_tensor("wv", [128, NCC * HD], BF16, kind="ExternalInput").ap()
    wo = nc.dram_tensor("wo", [128, G * C], BF16, kind="ExternalInput").ap()
    cos = nc.dram_tensor("cos", [HD, T], BF16, kind="ExternalInput").ap()
    sin = nc.dram_tensor("sin", [HD, T], BF16, kind="ExternalInput").ap()
    masks = nc.dram_tensor("masks", [128, 128], F32, kind="ExternalInput").ap()
    ident = nc.dram_tensor("ident", [128, 128], BF16, kind="ExternalInput").ap()
    onesfull = nc.dram_tensor("onesfull", [128, 128], F32, kind="ExternalInput").ap()
    y = nc.dram_tensor("y", [T, C], F32, kind="ExternalOutput").ap()

    with tile.TileContext(nc) as tc, ExitStack() as ctx:
        big = ctx.enter_context(tc.tile_pool(name="big", bufs=1))

        # persistent activations / weights
        qt_sb = big.tile([128, G * T], F32R)  # [d, h*T + t]
        kt_sb = big.tile([128, T], F32R)  # [d, t]
        v_sb = big.tile([128, NKT * HD], BF16)  # [t-part, kt*HD + d]
        wq_sb = big.tile([128, G * NCC * HD], BF16)  # [c-chunk p, j*2048 + ci*128 + d]
        wk_sb = big.tile([128, NCC * HD], BF16)
        wv_sb = big.tile([128, NCC * HD], BF16)
        cos_sb = big.tile([HD, T], BF16)
        sin_sb = big.tile([HD, T], BF16)
        wo_sb = big.tile([128, G * C], BF16)  # [j in head-chunk, h*C + c]
        mask_sb = big.tile([128, 128], F32)
        ident_sb = big.tile([128, 128], BF16)
        ones_sb = big.tile([128, 128], F32R)

        # rotating pools
        x_pool = ctx.enter_context(tc.tile_pool(name="xp", bufs=2))
        rp = ctx.enter_context(tc.tile_pool(name="rp", bufs=2))
        pt_pool = ctx.enter_context(tc.tile_pool(name="pt", bufs=8))
        pacc_pool = ctx.enter_context(tc.tile_pool(name="pacc", bufs=4))
        nrm_pool = ctx.enter_context(tc.tile_pool(name="nrm", bufs=2))
        ot_pool = ctx.enter_context(tc.tile_pool(name="ot", bufs=2))
        ysb_pool = ctx.enter_context(tc.tile_pool(name="ysb", bufs=2))

        # ---- upfront DMA issue, paced to the per-output projection order ----
        # scalar ring: wq head 0, wk, wv, wq heads 1-3, wo
        JW = NCC * HD  # 2048, per-head wq column count
        nc.scalar.dma_start(wq_sb[:, 0 : 4 * HD], wq[:, 0 : 4 * HD])
        nc.scalar.dma_start(wq_sb[:, 4 * HD : JW], wq[:, 4 * HD : JW])
        nc.scalar.dma_start(wk_sb[:], wk)
        nc.scalar.dma_start(wv_sb[:], wv)
        for j in range(1, G):
            nc.scalar.dma_start(wq_sb[:, j * JW : (j + 1) * JW], wq[:, j * JW : (j + 1) * JW])
        nc.scalar.dma_start(wo_sb[:], wo)
        # gpsimd ring: rope tables + constants
        nc.gpsimd.dma_start(cos_sb[:], cos[:])
        nc.gpsimd.dma_start(sin_sb[:], sin[:])
        nc.gpsimd.dma_start(mask_sb[:], masks[:])
        nc.gpsimd.dma_start(ident_sb[:], ident[:])
        nc.gpsimd.dma_start(ones_sb[:], onesfull.bitcast(F32R))

        x_tiles = {}

        def issue_x(tci, split=False):
            xt = x_pool.tile([128, NCC * TC], BF16, tag="x", name=f"x{tci}")
            base = tci * NCC * TC
            if split:
                # first chunk pieces race in on two rings in parallel
                for eng, lo, hi in (
                    (nc.sync, 0, 4),
                    (nc.sync, 4, 8),
                    (nc.gpsimd, 8, 12),
                    (nc.gpsimd, 12, 16),
                ):
                    eng.dma_start(
                        xt[:, lo * TC : hi * TC],
                        xh[:, base + lo * TC : base + hi * TC],
                    )
            else:
                nc.sync.dma_start(xt[:], xh[:, base : base + NCC * TC])
            x_tiles[tci] = xt

        issue_x(0, split=True)
        issue_x(1)

        # ---------------- projection segment for t-chunk tci ----------------
        def proj_segment(tci):
            ts = slice(tci * TC, (tci + 1) * TC)
            with ExitStack() as seg:
                qt_ps_pool = seg.enter_context(
                    tc.tile_pool(name=f"qtps{tci}", bufs=4, space="PSUM")
                )
                kv_ps_pool = seg.enter_context(
                    tc.tile_pool(name=f"kvps{tci}", bufs=2, space="PSUM")
                )
                tp_ps_pool = seg.enter_context(
                    tc.tile_pool(name=f"tpps{tci}", bufs=1, space="PSUM")
                )
                xt = x_tiles[tci]

                def contract(ps, w_sb, off):
                    for ci in range(NCC):
                        nc.tensor.matmul(
                            ps[:],
                            w_sb[:, off + ci * HD : off + (ci + 1) * HD],
                            xt[:, ci * TC : (ci + 1) * TC],
                            start=(ci == 0),
                            stop=(ci == NCC - 1),
                        )

                def raw_copy(key, ps, dtype=F32):
                    r = rp.tile(
                        [128, TC], dtype, tag="raw" + key[0], name=f"raw_{key}_{tci}"
                    )
                    nc.scalar.copy(r[:], ps[:])
                    return r

                # rope: out = q*cos + swap(q)*sin_rolled; the rotate-half is
                # fused into the sin-muls via partition-offset reads (sin_sb
                # is pre-rolled by 64 partitions on the host)
                def rope(raw, out_ap, eng):
                    t1 = rp.tile([128, TC], F32, tag="t1", name=f"t1_{out_ap.offset}")
                    eng.tensor_mul(t1[:], raw[:], cos_sb[:, ts])
                    t2 = rp.tile([128, TC], F32, tag="t2", name=f"t2_{out_ap.offset}")
                    eng.tensor_mul(t2[0:64, :], raw[64:128, :], sin_sb[64:128, ts])
                    eng.tensor_mul(t2[64:128, :], raw[0:64, :], sin_sb[0:64, ts])
                    eng.tensor_add(out_ap, t1[:], t2[:])

                def qslice(j):
                    return qt_sb[:, j * T + tci * TC : j * T + (tci + 1) * TC]

                # Q0 first: its rope gates the first attention S-matmuls
                q0_ps = qt_ps_pool.tile([128, TC], F32, tag="qtps", name=f"q0ps{tci}")
                contract(q0_ps, wq_sb, 0 * JW)
                rope(raw_copy("q0", q0_ps), qslice(0), nc.vector)

                kt_ps = kv_ps_pool.tile([128, TC], F32, tag="kvps", name=f"ktps{tci}")
                contract(kt_ps, wk_sb, 0)
                rope(raw_copy("k", kt_ps), kt_sb[:, ts], nc.vector)

                vt_ps = kv_ps_pool.tile([128, TC], F32, tag="kvps", name=f"vtps{tci}")
                contract(vt_ps, wv_sb, 0)
                vt_f = raw_copy("v", vt_ps, BF16)
                for s in range(TC // 128):
                    kt_i = tci * (TC // 128) + s
                    tp_ps = tp_ps_pool.tile([128, 128], BF16, tag="tp", name=f"tp{kt_i}")
                    nc.tensor.transpose(
                        tp_ps[:], vt_f[:, s * 128 : (s + 1) * 128], ident_sb[:]
                    )
                    nc.scalar.copy(v_sb[:, kt_i * HD : (kt_i + 1) * HD], tp_ps[:])

                for j, eng in ((1, nc.gpsimd), (2, nc.vector), (3, nc.gpsimd)):
                    qj_ps = qt_ps_pool.tile(
                        [128, TC], F32, tag="qtps", name=f"q{j}ps{tci}"
                    )
                    contract(qj_ps, wq_sb, j * JW)
                    rope(raw_copy(f"q{j}", qj_ps), qslice(j), eng)

        # ---------------- attention segment for q-block qb ----------------
        def attn_segment(qb):
            nkt = (qb + 1) * (TC // 128)
            if qb + 2 < NTC:
                issue_x(qb + 2)
            with ExitStack() as seg:
                st_pool = seg.enter_context(
                    tc.tile_pool(name=f"stps{qb}", bufs=6, space="PSUM")
                )
                ot_ps_pool = seg.enter_context(
                    tc.tile_pool(name=f"otps{qb}", bufs=2, space="PSUM")
                )
                ot_qb = ot_pool.tile([128, G * TC], BF16, tag="ot", name=f"ot{qb}")
                state = {}  # hg -> (pts, pacc, ot_ps)

                def emit_s(hg, kt, hh):
                    pts = state[hg][0]
                    dj = kt - 4 * qb
                    f0 = max(dj, 0) * 128
                    h = 2 * hg + hh
                    s_t = st_pool.tile(
                        [128, TC], F32, tag="st", name=f"st{qb}_{kt}_{h}"
                    )
                    nc.tensor.matmul(
                        s_t[:, f0:TC],
                        kt_sb[:, kt * 128 : (kt + 1) * 128],
                        qt_sb[:, h * T + qb * TC + f0 : h * T + (qb + 1) * TC],
                        start=True,
                        stop=True,
                    )
                    if dj >= 0:
                        nc.vector.tensor_add(
                            s_t[:, f0 : f0 + 128],
                            s_t[:, f0 : f0 + 128],
                            mask_sb[:],
                        )
                    pt = pt_pool.tile(
                        [128, TC], BF16, tag="pt", name=f"pt{qb}_{kt}_{h}"
                    )
                    nc.scalar.activation(
                        pt[:, f0:TC],
                        s_t[:, f0:TC],
                        mybir.ActivationFunctionType.Exp,
                    )
                    pts[(kt, hh)] = (pt, f0)

                def emit_acc(hg, kt, hh):
                    pts, pacc, ot_ps = state[hg]
                    st, sp = (kt == 0), (kt == nkt - 1)
                    if kt == 0:
                        ot_ps[hh] = ot_ps_pool.tile(
                            [128, TC], F32, tag="otps", name=f"otps{qb}_{hg}_{hh}"
                        )
                        pt, f0 = pts[(0, hh)]  # kept alive for the pair-init
                    elif kt == 1:
                        # init the running sum as pt0 + pt1 (no cast-copy)
                        pacc[hh] = pacc_pool.tile(
                            [128, TC], F32R, tag="pacc", name=f"pacc{qb}_{hg}_{hh}"
                        )
                        eng = nc.vector if hh == 0 else nc.gpsimd
                        pt0, g0 = pts.pop((0, hh))
                        pt, f0 = pts.pop((1, hh))
                        eng.tensor_add(
                            pacc[hh][:, f0:TC], pt0[:, f0:TC], pt[:, f0:TC]
                        )
                        if f0 > g0:
                            eng.tensor_copy(pacc[hh][:, g0:f0], pt0[:, g0:f0])
                    else:
                        pt, f0 = pts.pop((kt, hh))
                        eng = nc.vector if hh == 0 else nc.gpsimd
                        eng.tensor_add(
                            pacc[hh][:, f0:TC],
                            pacc[hh][:, f0:TC].bitcast(F32),
                            pt[:, f0:TC],
                        )
                    nc.tensor.matmul(
                        ot_ps[hh][:, f0:TC],
                        v_sb[:, kt * HD : (kt + 1) * HD],
                        pt[:, f0:TC],
                        start=st,
                        stop=sp,
                        skip_group_check=True,
                    )

                def epilogue(hg, hh):
                    _, pacc, ot_ps = state[hg]
                    h = 2 * hg + hh
                    sb_ps = st_pool.tile(
                        [128, TC], F32, tag="st", name=f"sps{qb}_{hg}_{hh}"
                    )
                    nc.tensor.matmul(
                        sb_ps[:], ones_sb[:], pacc[hh][:], start=True, stop=True
                    )
                    r_f = nrm_pool.tile([128, TC], F32, tag="rf", name=f"rf{qb}_{h}")
                    nc.vector.reciprocal_approx_fast(r_f[:], sb_ps[:])
                    nc.vector.tensor_mul(
                        ot_qb[:, h * TC : (h + 1) * TC], ot_ps[hh][:], r_f[:]
                    )

                for hg in range(G // 2):
                    state[hg] = ({}, [None, None], [None, None])
                    for kt in (0, 1):
                        for hh in range(2):
                            emit_s(hg, kt, hh)
                        if hg == 1 and kt == 0:
                            # hg0's epilogue hides behind hg1's first S-tiles
                            epilogue(0, 0)
                            epilogue(0, 1)
                    for kt in range(2, nkt):
                        for hh in range(2):
                            emit_s(hg, kt, hh)
                            emit_acc(hg, kt - 2, hh)
                    for hh in range(2):
                        emit_acc(hg, nkt - 2, hh)
                    for hh in range(2):
                        emit_acc(hg, nkt - 1, hh)

                # out-projection; hg1's epilogue hides behind the h0/h1
                # matmuls of the first row-tile (they only need hg0's norms)
                for tl in range(TC // 128):
                    tsub = qb * (TC // 128) + tl
                    ysb = ysb_pool.tile([128, C], F32, tag="ysb", name=f"ysb{tsub}")
                    y_ps = [
                        st_pool.tile([128, TC], F32, tag="st", name=f"yps{tsub}_{cc}")
                        for cc in range(C // TC)
                    ]
                    for h in range(G):
                        for cc in range(C // TC):
                            nc.tensor.matmul(
                                y_ps[cc][:],
                                ot_qb[:, h * TC + tl * 128 : h * TC + (tl + 1) * 128],
                                wo_sb[:, h * C + cc * TC : h * C + (cc + 1) * TC],
                                start=(h == 0),
                                stop=(h == G - 1),
                            )
                        if tl == 0 and h == 1:
                            epilogue(1, 0)
                            epilogue(1, 1)
                    for cc in range(C // TC):
                        ys = ysb[:, cc * TC : (cc + 1) * TC]
                        if cc % 2 == 0:
                            nc.vector.tensor_copy(ys, y_ps[cc][:])
                        else:
                            nc.scalar.copy(ys, y_ps[cc][:])
                        eng = nc.sync if cc % 2 == 0 else nc.scalar
                        eng.dma_start(
                            y[tsub * 128 : (tsub + 1) * 128, cc * TC : (cc + 1) * TC],
                            ys,
                        )

        for tci in range(NTC):
            proj_segment(tci)
            attn_segment(tci)

    nc.compile()
    return nc


def _rope_tables():
    theta = 1.0 / (10000.0 ** (np.arange(0, HD, 2, dtype=np.float32) / HD))
    freqs = np.arange(T, dtype=np.float32)[:, None] * theta[None, :]  # [T, 64]
    cos = np.concatenate([np.cos(freqs), np.cos(freqs)], axis=-1)  # [T, 128]
    sin = np.concatenate([np.sin(freqs), np.sin(freqs)], axis=-1)
    cosT = np.ascontiguousarray(cos.T).astype(np.float32)  # [128, T]
    sinT = np.ascontiguousarray(sin.T).astype(np.float32)
    sign = np.where(np.arange(HD) < HD // 2, np.float32(-1.0), np.float32(1.0))[:, None]
    sinT_signed = (sinT * sign).astype(np.float32)
    # rolled by 64 partitions: row d holds sin_signed[(d+64)%128] so the
    # rotate-half mul reads raw[d] and sin_sw[d] at the same base partition
    sinT_rolled = np.roll(sinT_signed, 64, axis=0).astype(np.float32)
    return cosT, sinT_rolled


def _masks():
    p = np.arange(128)[:, None]
    f = np.arange(128)[None, :]
    return np.where(p <= f, 0.0, MASK_NEG).astype(np.float32)


def _bf16(a):
    return np.ascontiguousarray(a).astype(ml_dtypes.bfloat16)


def _shuffle_w(w):
    # [NCC*128, F] -> [128, NCC*F]: partition p holds rows {p, 128+p, ...}
    ncc, f = w.shape[0] // 128, w.shape[1]
    return w.reshape(ncc, 128, f).transpose(1, 0, 2).reshape(128, ncc * f)


def _shuffle_wq(w):
    # [NCC*128, G*HD] -> [128, (j, ci, d)]
    return (
        w.reshape(NCC, 128, G, HD).transpose(1, 2, 0, 3).reshape(128, G * NCC * HD)
    )


def _shuffle_x(xb):
    # x[b] [T, C] -> [128, (tci, ci, tl)] with element (p,tci,ci,tl) =
    # x[tci*TC + tl, ci*128 + p]
    return (
        xb.reshape(NTC, TC, NCC, 128).transpose(3, 0, 2, 1).reshape(128, NTC * NCC * TC)
    )


def make_in_maps(x, Wq, Wk, Wv, Wo):
    x = np.asarray(x, dtype=np.float32)
    Wq = np.asarray(Wq, dtype=np.float32)
    Wk = np.asarray(Wk, dtype=np.float32)
    Wv = np.asarray(Wv, dtype=np.float32)
    Wo = np.asarray(Wo, dtype=np.float32)

    cosT, sinT = _rope_tables()
    masks = _masks()
    qscale = np.float32(1.0 / np.sqrt(HD))
    onesfull = np.ones((128, 128), dtype=np.float32)
    ident = np.eye(128, dtype=np.float32)

    xs = [_bf16(_shuffle_x(x[b])) for b in range(B)]
    in_maps = []
    for c in range(N_CORES):
        b, g = divmod(c, N_KV_HEADS)
        in_maps.append(
            {
                "xh": xs[b],
                "wq": _bf16(_shuffle_wq(Wq[:, g * GW : (g + 1) * GW] * qscale)),
                "wk": _bf16(_shuffle_w(Wk[:, g * HD : (g + 1) * HD])),
                "wv": _bf16(_shuffle_w(Wv[:, g * HD : (g + 1) * HD])),
                "wo": _bf16(_shuffle_w(Wo[g * GW : (g + 1) * GW, :])),
                "cos": _bf16(cosT),
                "sin": _bf16(sinT),
                "masks": masks,
                "ident": _bf16(ident),
                "onesfull": onesfull,
            }
        )
    return in_maps


def kernel(x, Wq, Wk, Wv, Wo):
    if "nc" not in _prog_cache:
        _prog_cache["nc"] = _build_program()
    nc = _prog_cache["nc"]

    in_maps = make_in_maps(x, Wq, Wk, Wv, Wo)
    res = run_bass_kernel_spmd(nc, in_maps, list(range(N_CORES)))
    _prog_cache["last_results"] = res

    out = np.zeros((B, T, C), dtype=np.float32)
    for c in range(N_CORES):
        b = c // N_KV_HEADS
        out[b] += res.results[c]["y"]
    return out


# revision 42
# speedup vs baseline: 1.0332x; 1.0332x over previous
"""Causal self-attention (GQA + RoPE) Trainium2 kernel.

Full-input contract: kernel(**inputs) takes the unsharded tensors and returns
the full [B, T, C] output. Internally shards over 8 NeuronCores as
(batch b in {0,1}) x (kv-head group g in {0..3}); each core computes the
attention output of its 4 query heads (one kv head) for its batch and the
partial out-projection against its 512 rows of Wo. The host sums the 4 group
partials per batch.

Per-core dataflow: projection (tci) and attention (qb=tci) segments are
interleaved; causality guarantees attention block qb only needs K/V/Q through
t-chunk qb. Projection runs one output at a time (Q0, K, V, Q1..Q3) so the
rope chain for the first attention head starts 1/6 into the segment instead
of at its end. Q/K stay f32r (fast DVE writes, f32r x f32r S-matmul);
x/weights/V/P/out-proj are bf16. Host pre-shuffles all inputs into SBUF
partition layout so every DMA is a contiguous 2D pattern, spread over the
sync / scalar / gpsimd DMA rings to match when each tensor is needed.
Softmax denominators are running sums on DVE/GpSimd (pacc) + one
ones-matmul per (head, q-block). S-matmuls are software-pipelined one k-tile
ahead of AV per head so exp latency stays off the PE critical path. The
rotate-half is fused into the rope sin-muls via partition-offset reads of a
host-rolled sin table.
"""

import sys

for _p in ("/opt/trn_rl_repo", "/root/.axon_site/_ro/trn_rl_repo"):
    if _p not in sys.path:
        sys.path.append(_p)

import numpy as np
import ml_dtypes
from contextlib import ExitStack

import concourse.bass as bass
import concourse.bacc as bacc
import concourse.tile as tile
import concourse.mybir as mybir
from concourse.bass_utils import run_bass_kernel_spmd

F32 = mybir.dt.float32
F32R = mybir.dt.float32r
BF16 = mybir.dt.bfloat16

B, T, C = 2, 2048, 2048
N_HEADS, N_KV_HEADS, HD = 16, 4, 128
G = N_HEADS // N_KV_HEADS  # heads per group = 4
GW = G * HD  # 512, per-group Q width / Wo row count
N_CORES = 8
TC = 512  # q-block width
NTC = T // TC  # 4
NKT = T // HD  # 16 k-tiles of 128
NCC = C // 128  # 16 contraction chunks
MASK_NEG = -1.0e30

_prog_cache = {}


def _build_program():
    nc = bacc.Bacc(
        "TRN2",
        target_bir_lowering=False,
        debug=False,
        enable_asserts=False,
        num_devices=N_CORES,
    )

    # all host tensors pre-shuffled to SBUF layout (partition dim first);
    # wq additionally grouped by head: [128, (j, ci, d)]
    xh = nc.dram_tensor("xh", [128, NTC * NCC * TC], BF16, kind="ExternalInput").ap()
    wq = nc.dram_tensor("wq", [128, G * NCC * HD], BF16, kind="ExternalInput").ap()
    wk = nc.dram_tensor("wk", [128, NCC * HD], BF16, kind="ExternalInput").ap()
    wv = nc.dram_tensor("wv", [128, NCC * HD], BF16, kind="ExternalInput").ap()
    wo = nc.dram_tensor("wo", [128, G * C], BF16, kind="ExternalInput").ap()
    cos = nc.dram_tensor("cos", [HD, T], BF16, kind="ExternalInput").ap()
    sin = nc.dram_tensor("sin", [HD, T], BF16, kind="ExternalInput").ap()
    masks = nc.dram_tensor("masks", [128, 128], F32, kind="ExternalInput").ap()
    ident = nc.dram_tensor("ident", [128, 128], BF16, kind="ExternalInput").ap()
    onesfull = nc.dram_tensor("onesfull", [128, 128], F32, kind="ExternalInput").ap()
    y = nc.dram_tensor("y", [T, C], F32, kind="ExternalOutput").ap()

    with tile.TileContext(nc) as tc, ExitStack() as ctx:
        big = ctx.enter_context(tc.tile_pool(name="big", bufs=1))

        # persistent activations / weights
        qt_sb = big.tile([128, G * T], F32R)  # [d, h*T + t]
        kt_sb = big.tile([128, T], F32R)  # [d, t]
        v_sb = big.tile([128, NKT * HD], BF16)  # [t-part, kt*HD + d]
        wq_sb = big.tile([128, G * NCC * HD], BF16)  # [c-chunk p, j*2048 + ci*128 + d]
        wk_sb = big.tile([128, NCC * HD], BF16)
        wv_sb = big.tile([128, NCC * HD], BF16)
        cos_sb = big.tile([HD, T], BF16)
        sin_sb = big.tile([HD, T], BF16)
        wo_sb = big.tile([128, G * C], BF16)  # [j in head-chunk, h*C + c]
        mask_sb = big.tile([128, 128], F32)
        ident_sb = big.tile([128, 128], BF16)
        ones_sb = big.tile([128, 128], F32R)

        # rotating pools
        x_pool = ctx.enter_context(tc.tile_pool(name="xp", bufs=2))
        rp = ctx.enter_context(tc.tile_pool(name="rp", bufs=2))
        pt_pool = ctx.enter_context(tc.tile_pool(name="pt", bufs=8))
        pacc_pool = ctx.enter_context(tc.tile_pool(name="pacc", bufs=4))
        nrm_pool = ctx.enter_context(tc.tile_pool(name="nrm", bufs=2))
        ot_pool = ctx.enter_context(tc.tile_pool(name="ot", bufs=2))
        ysb_pool = ctx.enter_context(tc.tile_pool(name="ysb", bufs=2))

        # ---- upfront DMA issue, paced to the per-output projection order ----
        # scalar ring: wq head 0, wk, wv, wq heads 1-3, wo
        JW = NCC * HD  # 2048, per-head wq column count
        nc.scalar.dma_start(wq_sb[:, 0 : 4 * HD], wq[:, 0 : 4 * HD])
        nc.scalar.dma_start(wq_sb[:, 4 * HD : JW], wq[:, 4 * HD : JW])
        nc.scalar.dma_start(wk_sb[:], wk)
        nc.scalar.dma_start(wv_sb[:], wv)
        for j in range(1, G):
            nc.scalar.dma_start(wq_sb[:, j * JW : (j + 1) * JW], wq[:, j * JW : (j + 1) * JW])
        nc.scalar.dma_start(wo_sb[:], wo)
        # gpsimd ring: rope tables + constants
        nc.gpsimd.dma_start(cos_sb[:], cos[:])
        nc.gpsimd.dma_start(sin_sb[:], sin[:])
        nc.gpsimd.dma_start(mask_sb[:], masks[:])
        nc.gpsimd.dma_start(ident_sb[:], ident[:])
        nc.gpsimd.dma_start(ones_sb[:], onesfull.bitcast(F32R))

        x_tiles = {}

        def issue_x(tci, split=False):
            xt = x_pool.tile([128, NCC * TC], BF16, tag="x", name=f"x{tci}")
            base = tci * NCC * TC
            if split:
                for lo, hi in ((0, 4), (4, 8), (8, 16)):
                    nc.sync.dma_start(
                        xt[:, lo * TC : hi * TC],
                        xh[:, base + lo * TC : base + hi * TC],
                    )
            else:
                nc.sync.dma_start(xt[:], xh[:, base : base + NCC * TC])
            x_tiles[tci] = xt

        issue_x(0, split=True)
        issue_x(1)

        # ---------------- projection segment for t-chunk tci ----------------
        def proj_segment(tci):
            ts = slice(tci * TC, (tci + 1) * TC)
            with ExitStack() as seg:
                qt_ps_pool = seg.enter_context(
                    tc.tile_pool(name=f"qtps{tci}", bufs=4, space="PSUM")
                )
                kv_ps_pool = seg.enter_context(
                    tc.tile_pool(name=f"kvps{tci}", bufs=2, space="PSUM")
                )
                tp_ps_pool = seg.enter_context(
                    tc.tile_pool(name=f"tpps{tci}", bufs=1, space="PSUM")
                )
                xt = x_tiles[tci]

                def contract(ps, w_sb, off):
                    for ci in range(NCC):
                        nc.tensor.matmul(
                            ps[:],
                            w_sb[:, off + ci * HD : off + (ci + 1) * HD],
                            xt[:, ci * TC : (ci + 1) * TC],
                            start=(ci == 0),
                            stop=(ci == NCC - 1),
                        )

                def raw_copy(key, ps, dtype=F32):
                    r = rp.tile(
                        [128, TC], dtype, tag="raw" + key[0], name=f"raw_{key}_{tci}"
                    )
                    nc.scalar.copy(r[:], ps[:])
                    return r

                # rope: out = q*cos + swap(q)*sin_rolled; the rotate-half is
                # fused into the sin-muls via partition-offset reads (sin_sb
                # is pre-rolled by 64 partitions on the host)
                def rope(raw, out_ap, eng):
                    t1 = rp.tile([128, TC], F32, tag="t1", name=f"t1_{out_ap.offset}")
                    eng.tensor_mul(t1[:], raw[:], cos_sb[:, ts])
                    t2 = rp.tile([128, TC], F32, tag="t2", name=f"t2_{out_ap.offset}")
                    eng.tensor_mul(t2[0:64, :], raw[64:128, :], sin_sb[64:128, ts])
                    eng.tensor_mul(t2[64:128, :], raw[0:64, :], sin_sb[0:64, ts])
                    eng.tensor_add(out_ap, t1[:], t2[:])

                def qslice(j):
                    return qt_sb[:, j * T + tci * TC : j * T + (tci + 1) * TC]

                # Q0 first: its rope gates the first attention S-matmuls
                q0_ps = qt_ps_pool.tile([128, TC], F32, tag="qtps", name=f"q0ps{tci}")
                contract(q0_ps, wq_sb, 0 * JW)
                rope(raw_copy("q0", q0_ps), qslice(0), nc.vector)

                kt_ps = kv_ps_pool.tile([128, TC], F32, tag="kvps", name=f"ktps{tci}")
                contract(kt_ps, wk_sb, 0)
                rope(raw_copy("k", kt_ps), kt_sb[:, ts], nc.vector)

                vt_ps = kv_ps_pool.tile([128, TC], F32, tag="kvps", name=f"vtps{tci}")
                contract(vt_ps, wv_sb, 0)
                vt_f = raw_copy("v", vt_ps, BF16)
                for s in range(TC // 128):
                    kt_i = tci * (TC // 128) + s
                    tp_ps = tp_ps_pool.tile([128, 128], BF16, tag="tp", name=f"tp{kt_i}")
                    nc.tensor.transpose(
                        tp_ps[:], vt_f[:, s * 128 : (s + 1) * 128], ident_sb[:]
                    )
                    nc.scalar.copy(v_sb[:, kt_i * HD : (kt_i + 1) * HD], tp_ps[:])

                for j, eng in ((1, nc.gpsimd), (2, nc.vector), (3, nc.gpsimd)):
                    qj_ps = qt_ps_pool.tile(
                        [128, TC], F32, tag="qtps", name=f"q{j}ps{tci}"
                    )
                    contract(qj_ps, wq_sb, j * JW)
                    rope(raw_copy(f"q{j}", qj_ps), qslice(j), eng)

        # ---------------- attention segment for q-block qb ----------------
        def attn_segment(qb):
            nkt = (qb + 1) * (TC // 128)
            if qb + 2 < NTC:
                issue_x(qb + 2)
            with ExitStack() as seg:
                st_pool = seg.enter_context(
                    tc.tile_pool(name=f"stps{qb}", bufs=6, space="PSUM")
                )
                ot_ps_pool = seg.enter_context(
                    tc.tile_pool(name=f"otps{qb}", bufs=2, space="PSUM")
                )
                ot_qb = ot_pool.tile([128, G * TC], BF16, tag="ot", name=f"ot{qb}")
                state = {}  # hg -> (pts, pacc, ot_ps)

                def emit_s(hg, kt, hh):
                    pts = state[hg][0]
                    dj = kt - 4 * qb
                    f0 = max(dj, 0) * 128
                    h = 2 * hg + hh
                    s_t = st_pool.tile(
                        [128, TC], F32, tag="st", name=f"st{qb}_{kt}_{h}"
                    )
                    nc.tensor.matmul(
                        s_t[:, f0:TC],
                        kt_sb[:, kt * 128 : (kt + 1) * 128],
                        qt_sb[:, h * T + qb * TC + f0 : h * T + (qb + 1) * TC],
                        start=True,
                        stop=True,
                    )
                    if dj >= 0:
                        nc.vector.tensor_add(
                            s_t[:, f0 : f0 + 128],
                            s_t[:, f0 : f0 + 128],
                            mask_sb[:],
                        )
                    pt = pt_pool.tile(
                        [128, TC], BF16, tag="pt", name=f"pt{qb}_{kt}_{h}"
                    )
                    nc.scalar.activation(
                        pt[:, f0:TC],
                        s_t[:, f0:TC],
                        mybir.ActivationFunctionType.Exp,
                    )
                    pts[(kt, hh)] = (pt, f0)

                def emit_acc(hg, kt, hh):
                    pts, pacc, ot_ps = state[hg]
                    st, sp = (kt == 0), (kt == nkt - 1)
                    if kt == 0:
                        ot_ps[hh] = ot_ps_pool.tile(
                            [128, TC], F32, tag="otps", name=f"otps{qb}_{hg}_{hh}"
                        )
                        pt, f0 = pts[(0, hh)]  # kept alive for the pair-init
                    elif kt == 1:
                        # init the running sum as pt0 + pt1 (no cast-copy)
                        pacc[hh] = pacc_pool.tile(
                            [128, TC], F32R, tag="pacc", name=f"pacc{qb}_{hg}_{hh}"
                        )
                        pt0, g0 = pts.pop((0, hh))
                        pt, f0 = pts.pop((1, hh))
                        nc.vector.tensor_add(
                            pacc[hh][:, f0:TC], pt0[:, f0:TC], pt[:, f0:TC]
                        )
                        if f0 > g0:
                            nc.vector.tensor_copy(pacc[hh][:, g0:f0], pt0[:, g0:f0])
                    else:
                        pt, f0 = pts.pop((kt, hh))
                        nc.vector.tensor_add(
                            pacc[hh][:, f0:TC],
                            pacc[hh][:, f0:TC].bitcast(F32),
                            pt[:, f0:TC],
                        )
                    nc.tensor.matmul(
                        ot_ps[hh][:, f0:TC],
                        v_sb[:, kt * HD : (kt + 1) * HD],
                        pt[:, f0:TC],
                        start=st,
                        stop=sp,
                        skip_group_check=True,
                    )

                def epilogue(hg, hh):
                    _, pacc, ot_ps = state[hg]
                    h = 2 * hg + hh
                    sb_ps = st_pool.tile(
                        [128, TC], F32, tag="st", name=f"sps{qb}_{hg}_{hh}"
                    )
                    nc.tensor.matmul(
                        sb_ps[:], ones_sb[:], pacc[hh][:], start=True, stop=True
                    )
                    r_f = nrm_pool.tile([128, TC], F32, tag="rf", name=f"rf{qb}_{h}")
                    nc.vector.reciprocal_approx_fast(r_f[:], sb_ps[:])
                    nc.vector.tensor_mul(
                        ot_qb[:, h * TC : (h + 1) * TC], ot_ps[hh][:], r_f[:]
                    )

                for hg in range(G // 2):
                    state[hg] = ({}, [None, None], [None, None])
                    for kt in (0, 1):
                        for hh in range(2):
                            emit_s(hg, kt, hh)
                        if hg == 1 and kt == 0:
                            # hg0's epilogue hides behind hg1's first S-tiles
                            epilogue(0, 0)
                            epilogue(0, 1)
                    for kt in range(2, nkt):
                        for hh in range(2):
                            emit_s(hg, kt, hh)
                            emit_acc(hg, kt - 2, hh)
                    for hh in range(2):
                        emit_acc(hg, nkt - 2, hh)
                    for hh in range(2):
                        emit_acc(hg, nkt - 1, hh)

                # out-projection; hg1's epilogue hides behind the h0/h1
                # matmuls of the first row-tile (they only need hg0's norms)
                for tl in range(TC // 128):
                    tsub = qb * (TC // 128) + tl
                    ysb = ysb_pool.tile([128, C], F32, tag="ysb", name=f"ysb{tsub}")
                    y_ps = [
                        st_pool.tile([128, TC], F32, tag="st", name=f"yps{tsub}_{cc}")
                        for cc in range(C // TC)
                    ]
                    for h in range(G):
                        for cc in range(C // TC):
                            nc.tensor.matmul(
                                y_ps[cc][:],
                                ot_qb[:, h * TC + tl * 128 : h * TC + (tl + 1) * 128],
                                wo_sb[:, h * C + cc * TC : h * C + (cc + 1) * TC],
                                start=(h == 0),
                                stop=(h == G - 1),
                            )
                        if tl == 0 and h == 0:
                            epilogue(1, 0)
                            epilogue(1, 1)
                    for cc in range(C // TC):
                        ys = ysb[:, cc * TC : (cc + 1) * TC]
                        if cc % 2 == 0:
                            nc.vector.tensor_copy(ys, y_ps[cc][:])
                        else:
                            nc.scalar.copy(ys, y_ps[cc][:])
                        eng = nc.sync if cc % 2 == 0 else nc.scalar
                        eng.dma_start(
                            y[tsub * 128 : (tsub + 1) * 128, cc * TC : (cc + 1) * TC],
                            ys,
                        )

        for tci in range(NTC):
            proj_segment(tci)
            attn_segment(tci)

    nc.compile()
    return nc


def _rope_tables():
    theta = 1.0 / (10000.0 ** (np.arange(0, HD, 2, dtype=np.float32) / HD))
    freqs = np.arange(T, dtype=np.float32)[:, None] * theta[None, :]  # [T, 64]
    cos = np.concatenate([np.cos(freqs), np.cos(freqs)], axis=-1)  # [T, 128]
    sin = np.concatenate([np.sin(freqs), np.sin(freqs)], axis=-1)
    cosT = np.ascontiguousarray(cos.T).astype(np.float32)  # [128, T]
    sinT = np.ascontiguousarray(sin.T).astype(np.float32)
    sign = np.where(np.arange(HD) < HD // 2, np.float32(-1.0), np.float32(1.0))[:, None]
    sinT_signed = (sinT * sign).astype(np.float32)
    # rolled by 64 partitions: row d holds sin_signed[(d+64)%128] so the
    # rotate-half mul reads raw[d] and sin_sw[d] at the same base partition
    sinT_rolled = np.roll(sinT_signed, 64, axis=0).astype(np.float32)
    return cosT, sinT_rolled


def _masks():
    p = np.arange(128)[:, None]
    f = np.arange(128)[None, :]
    return np.where(p <= f, 0.0, MASK_NEG).astype(np.float32)


def _bf16(a):
    return np.ascontiguousarray(a).astype(ml_dtypes.bfloat16)


def _shuffle_w(w):
    # [NCC*128, F] -> [128, NCC*F]: partition p holds rows {p, 128+p, ...}
    ncc, f = w.shape[0] // 128, w.shape[1]
    return w.reshape(ncc, 128, f).transpose(1, 0, 2).reshape(128, ncc * f)


def _shuffle_wq(w):
    # [NCC*128, G*HD] -> [128, (j, ci, d)]
    return (
        w.reshape(NCC, 128, G, HD).transpose(1, 2, 0, 3).reshape(128, G * NCC * HD)
    )


def _shuffle_x(xb):
    # x[b] [T, C] -> [128, (tci, ci, tl)] with element (p,tci,ci,tl) =
    # x[tci*TC + tl, ci*128 + p]
    return (
        xb.reshape(NTC, TC, NCC, 128).transpose(3, 0, 2, 1).reshape(128, NTC * NCC * TC)
    )


def make_in_maps(x, Wq, Wk, Wv, Wo):
    x = np.asarray(x, dtype=np.float32)
    Wq = np.asarray(Wq, dtype=np.float32)
    Wk = np.asarray(Wk, dtype=np.float32)
    Wv = np.asarray(Wv, dtype=np.float32)
    Wo = np.asarray(Wo, dtype=np.float32)

    cosT, sinT = _rope_tables()
    masks = _masks()
    qscale = np.float32(1.0 / np.sqrt(HD))
    onesfull = np.ones((128, 128), dtype=np.float32)
    ident = np.eye(128, dtype=np.float32)

    xs = [_bf16(_shuffle_x(x[b])) for b in range(B)]
    in_maps = []
    for c in range(N_CORES):
        b, g = divmod(c, N_KV_HEADS)
        in_maps.append(
            {
                "xh": xs[b],
                "wq": _bf16(_shuffle_wq(Wq[:, g * GW : (g + 1) * GW] * qscale)),
                "wk": _bf16(_shuffle_w(Wk[:, g * HD : (g + 1) * HD])),
                "wv": _bf16(_shuffle_w(Wv[:, g * HD : (g + 1) * HD])),
                "wo": _bf16(_shuffle_w(Wo[g * GW : (g + 1) * GW, :])),
                "cos": _bf16(cosT),
                "sin": _bf16(sinT),
                "masks": masks,
                "ident": _bf16(ident),
                "onesfull": onesfull,
            }
        )
    return in_maps


def kernel(x, Wq, Wk, Wv, Wo):
    if "nc" not in _prog_cache:
        _prog_cache["nc"] = _build_program()
    nc = _prog_cache["nc"]

    in_maps = make_in_maps(x, Wq, Wk, Wv, Wo)
    res = run_bass_kernel_spmd(nc, in_maps, list(range(N_CORES)))
    _prog_cache["last_results"] = res

    out = np.zeros((B, T, C), dtype=np.float32)
    for c in range(N_CORES):
        b = c // N_KV_HEADS
        out[b] += res.results[c]["y"]
    return out
